# revision 2
# baseline (speedup 1.0000x reference)
"""Trainium2 Bass kernel for MetaDynamics potential evaluation.

out[p] = sum_h hgt[h] * exp(-0.5 * sum_d (cen[h,d]-col[p,d])^2 / wdt[h,d]^2)
with H=16384 hills, P=4096 points, D=8 CVs, hills sharded over 8 cores
(2048 hills/core); every core computes a partial [4096] potential and the
host sums the partials.

e' comes from the rank-17 quadratic-form expansion as a K=51 bf16
hi/lo-split matmul. The weights are pre-scaled so the matmul directly
produces u[h,p] = (2^23/ln2)*e'[h,p] + 127*2^23.

The exp work per 128-point tile is split across TWO engines (the ACT
engine alone, at 1 exp/lane/cycle, was the ~64us/core bottleneck):
  - ACT: exact exp of CA=1536 hills using activation's free scale/bias
    (scale=ln2/2^23, bias=-127*ln2) + fused accumulate (accum_out).
  - DVE: Schraudolph bit-trick exp of CD=512 hills in 2 passes:
      p1: zi = int32(max(u, 0))   (the int BITS of clamped u ARE the
          linear-mantissa approximation of 2^(e'*log2e) = exp(e'))
      p2: acc += sum(bitcast_f32(zi) * g)   (g = constant debias)
    Per-element rel err ~1.8% rms after debias, applied to 25% of hills,
    averaged over thousands of hills per point -> ~2.5e-4 global
    (the gate is 2e-2).

Each consumer gets its OWN psum tile (pd: 1 bank for DVE, pa: 3 banks
for ACT) and its own accumulator tile, so the engines never serialize on
false tile dependencies.
"""

import numpy as np
import ml_dtypes

import concourse.bacc as bacc
import concourse.mybir as mybir
import concourse.tile as tile
from concourse import bass_utils

H, P, D = 16384, 4096, 8
NCORES = 8
HL = H // NCORES          # hills per core
K = 51                    # 3 x 17 stacked hi/lo blocks
PT = 128                  # points per tile (PSUM partitions)
NPT = P // PT             # 32 p-tiles
MMN = 512                 # moving-operand cols per matmul
CD = 512                  # hills per tile on DVE (schraudolph): 1 psum bank
CA = HL - CD              # hills per tile on ACT (exact): 3 psum banks

A_SCALE = float(2**23 / np.log(2.0))     # u = A*e' + B
B_CONST = float(127.0 * 2**23)
ACT_SCALE = float(1.0 / A_SCALE)
ACT_BIAS = float(-B_CONST / A_SCALE)     # = -127*ln2
G_DEBIAS = 0.96067                       # E[rho]/E[rho^2], rho=(1+m)/2^m

BF16 = mybir.dt.bfloat16
F32 = mybir.dt.float32
I32 = mybir.dt.int32

_NC_CACHE = None


def _build_nc():
    nc = bacc.Bacc(
        "TRN2",
        target_bir_lowering=False,
        debug=False,
        enable_asserts=False,
        num_devices=NCORES,
    )
    ft = nc.dram_tensor("ft", [K, P], BF16, kind="ExternalInput").ap()
    wt = nc.dram_tensor("wt", [K, HL], BF16, kind="ExternalInput").ap()
    misc = nc.dram_tensor("misc", [PT, 1], F32, kind="ExternalInput").ap()
    # out columns: [0:NPT] ACT partials, [NPT:2*NPT] DVE partials
    out = nc.dram_tensor("out", [PT, 2 * NPT], F32, kind="ExternalOutput").ap()

    with tile.TileContext(nc) as tc:
        with (
            tc.tile_pool(name="const", bufs=1) as cpool,
            tc.tile_pool(name="zi", bufs=2) as zpool,
            tc.tile_pool(name="psd", bufs=2, space="PSUM") as ppd,
            tc.tile_pool(name="psa", bufs=2, space="PSUM") as ppa,
        ):
            ftt = cpool.tile([K, P], BF16)
            wtt = cpool.tile([K, HL], BF16)
            biast = cpool.tile([PT, 1], F32)
            acc_a = cpool.tile([PT, NPT], F32)
            acc_d = cpool.tile([PT, NPT], F32)
            scratch = cpool.tile([PT, CD], F32)
            escr = cpool.tile([PT, CA], F32)

            # Startup DMAs: wt halves on the gpsimd queue (idle early), ft
            # staged in chunks on the sync queue, bias on the scalar queue.
            nc.gpsimd.dma_start(wtt[:, 0 : HL // 2], wt[:, 0 : HL // 2])
            nc.gpsimd.dma_start(wtt[:, HL // 2 : HL], wt[:, HL // 2 : HL])
            nc.scalar.dma_start(biast[:], misc[:])
            nc.sync.dma_start(ftt[:, 0:PT], ft[:, 0:PT])
            nc.sync.dma_start(ftt[:, PT:1024], ft[:, PT:1024])
            nc.sync.dma_start(ftt[:, 1024:2432], ft[:, 1024:2432])
            nc.sync.dma_start(ftt[:, 2432:P], ft[:, 2432:P])

            for i in range(NPT):
                # separate psum tiles per consumer: dependencies decouple
                pd = ppd.tile([PT, CD], F32)   # 1 PSUM bank, read by DVE
                pa = ppa.tile([PT, CA], F32)   # 3 PSUM banks, read by ACT
                nc.tensor.matmul(
                    pd[:],
                    lhsT=ftt[:, i * PT : (i + 1) * PT],
                    rhs=wtt[:, 0:MMN],
                    start=True,
                    stop=True,
                )
                for j in range(CA // MMN):
                    nc.tensor.matmul(
                        pa[:, j * MMN : (j + 1) * MMN],
                        lhsT=ftt[:, i * PT : (i + 1) * PT],
                        rhs=wtt[:, CD + j * MMN : CD + (j + 1) * MMN],
                        start=True,
                        stop=True,
                    )
                # DVE: schraudolph exp on the first CD hills
                zi = zpool.tile([PT, CD], I32)
                nc.vector.tensor_scalar(
                    zi[:], pd[:], 0.0, None, mybir.AluOpType.max
                )
                # ACT: exact exp + fused accumulate on the last CA hills
                nc.scalar.activation(
                    escr[:],
                    pa[:],
                    mybir.ActivationFunctionType.Exp,
                    scale=ACT_SCALE,
                    bias=biast[:, 0:1],
                    accum_out=acc_a[:, i : i + 1],
                )
                nc.vector.tensor_scalar(
                    scratch[:],
                    zi[:].bitcast(F32),
                    G_DEBIAS,
                    0.0,
                    mybir.AluOpType.mult,
                    mybir.AluOpType.add,
                    accum_out=acc_d[:, i : i + 1],
                )
                if i == NPT // 2 - 1:
                    nc.sync.dma_start(out[:, : NPT // 2], acc_a[:, : NPT // 2])
                    nc.sync.dma_start(
                        out[:, NPT : NPT + NPT // 2], acc_d[:, : NPT // 2]
                    )
            nc.sync.dma_start(out[:, NPT // 2 : NPT], acc_a[:, NPT // 2 :])
            nc.sync.dma_start(out[:, NPT + NPT // 2 :], acc_d[:, NPT // 2 :])

    nc.compile()
    return nc


def _get_nc():
    global _NC_CACHE
    if _NC_CACHE is None:
        _NC_CACHE = _build_nc()
    return _NC_CACHE


def _split_bf16(x64):
    hi = x64.astype(ml_dtypes.bfloat16)
    lo = (x64 - hi.astype(np.float64)).astype(ml_dtypes.bfloat16)
    return hi, lo


def _prepare_inputs(col, cen, wdt, hgt):
    col64 = col.astype(np.float64)
    cen64 = cen.astype(np.float64)
    wdt64 = wdt.astype(np.float64)
    hgt64 = np.maximum(hgt.astype(np.float64), 1e-38)

    c = 1.0 / (wdt64 * wdt64)                                     # [H, D]
    a = np.sum(cen64 * cen64 * c, axis=1) - 2.0 * np.log(hgt64)   # [H]
    W = np.concatenate([cen64 * c, -0.5 * c, -0.5 * a[:, None]], axis=1)  # [H,17]
    W = W * A_SCALE
    W[:, 16] += B_CONST
    F = np.concatenate([col64, col64 * col64, np.ones((P, 1))], axis=1)   # [P,17]

    Whi, Wlo = _split_bf16(W)
    Fhi, Flo = _split_bf16(F)

    ft = np.ascontiguousarray(np.concatenate([Fhi.T, Flo.T, Fhi.T], axis=0))  # [51,P]
    wt_full = np.concatenate([Whi.T, Whi.T, Wlo.T], axis=0)                   # [51,H]
    wts = [
        np.ascontiguousarray(wt_full[:, i * HL : (i + 1) * HL]) for i in range(NCORES)
    ]
    misc = np.full((PT, 1), ACT_BIAS, dtype=np.float32)
    return ft, wts, misc


def run_on_hw(col, cen, wdt, hgt, trace=False):
    """Run the SPMD kernel on 8 cores; returns (out[P] f32, BassKernelResults)."""
    ft, wts, misc = _prepare_inputs(col, cen, wdt, hgt)
    nc = _get_nc()
    in_maps = [{"ft": ft, "wt": wts[i], "misc": misc} for i in range(NCORES)]
    res = bass_utils.run_bass_kernel_spmd(
        nc, in_maps, core_ids=list(range(NCORES)), trace=trace
    )
    total = np.zeros(P, dtype=np.float64)
    for r in res.results:
        o = r["out"].astype(np.float64)
        total += (o[:, :NPT] + o[:, NPT:]).T.reshape(P)
    return total.astype(np.float32), res


def kernel(col, cen, wdt, hgt):
    out, _ = run_on_hw(col, cen, wdt, hgt, trace=False)
    return out
